# revision 37
# baseline (speedup 1.0000x reference)
"""Trainium2 Bass kernel for the few-shot knn-attention module.

Pipeline per sample (512 ch, 16x16 spatial):
  mask = softmax_{c,h,w}(W @ x); xm = x * mask  (mask kept unnormalized on
  device; the softmax denominator is folded algebraically; b==0 by spec)
  prototypes s = mean over 5 shots+space of xm; queries scored by cosine
  similarity against s; softmax over classes; mean over space -> (75, 5).

Distribution: data-parallel over the 100 samples on 8 NeuronCores with 13
slots/core (3 shot slots + 1 shared shot/query slot + 9 query slots,
zero-padded).  All 1x1 convs run in fp8 DoubleRow (W pre-scaled by 16,
folded back in the exp); x stays bf16 for the mask product so the
cosine path keeps full precision.  The prototype partial sums are
exchanged with an AllGather + local tree-sum (modeled ~1.8x cheaper
than AllReduce); the score matmuls are emitted "pre-transposed"
(stationary=xm chunk, moving=prototypes) so scores and |q|^2 land in
[spatial-position, class] orientation directly - no PSUM->SBUF copies
or PE transposes in the tail.
"""

import numpy as np
import ml_dtypes

import concourse.bass as bass
import concourse.mybir as mybir
import concourse.tile as tile
from concourse import bacc
from concourse.bass_utils import run_bass_kernel_spmd

# Force the act-table chooser onto the one set containing BOTH Exp and Ln
# ("natural_log_exp_and_others") so the kernel pays a single table load
# instead of swapping between the exp-only and ln-only sets (~2.7us each).
import concourse.hw_specs as _hw_specs

_ORIG_GET_ACT_TABLES = _hw_specs.get_activation_tables


def _nl_exp_only_tables(arch):
    t = _ORIG_GET_ACT_TABLES(arch)
    return {
        k: (v if k == "natural_log_exp_and_others" else set()) for k, v in t.items()
    }


bacc.get_activation_tables = _nl_exp_only_tables

N_CORES = 8
WAY = 5
SHOT = 5
C = 512
F = 256  # 16*16
KO = C // 128  # 4 partition tiles of the channel dim
NSHOT_SLOTS = 4   # slots 0..3 feed the prototype partials (slot 3 shared)
NQ_SLOTS = 10     # slots 3..12 are query-capable (slot 3 shared)
NSLOTS = 13
NPACKS = NQ_SLOTS // 2
W8_SCALE = 16.0   # W is pre-scaled by 16 for fp8; folded back in the exp
LN10 = float(np.log(10.0))

F32 = mybir.dt.float32
BF16 = mybir.dt.bfloat16
FP8 = mybir.dt.float8e4
EXP = mybir.ActivationFunctionType.Exp
LN = mybir.ActivationFunctionType.Ln
DR = mybir.MatmulPerfMode.DoubleRow
MULT = mybir.AluOpType.mult
ADD = mybir.AluOpType.add


def build_nc(kind="AllGather"):
    nc = bacc.Bacc(None, target_bir_lowering=False)
    xs = nc.dram_tensor("xs", [NSLOTS, C, F], BF16, kind="ExternalInput")
    xs8 = nc.dram_tensor("xs8", [NSLOTS, C, F], FP8, kind="ExternalInput")
    wt8 = nc.dram_tensor("wt8", [C, C], FP8, kind="ExternalInput")
    sw = nc.dram_tensor("sw", [NSHOT_SLOTS, 128, WAY], F32, kind="ExternalInput")
    out = nc.dram_tensor("out", [1, NQ_SLOTS * WAY], F32, kind="ExternalOutput")

    with tile.TileContext(nc) as tc:
        with (
            tc.tile_pool(name="singles", bufs=1) as singles,
            tc.tile_pool(name="xepool", bufs=3) as xepool,
            tc.tile_pool(name="xmq", bufs=NQ_SLOTS) as xmq_pool,
            tc.tile_pool(name="xms", bufs=2) as xms_pool,
            tc.tile_pool(name="xm2", bufs=3) as xm2_pool,
            tc.tile_pool(name="work", bufs=4) as work,
            tc.tile_pool(name="pconv", bufs=2, space="PSUM") as pconv,
            tc.tile_pool(name="pscratch", bufs=2, space="PSUM") as pscratch,
            tc.tile_pool(name="pscores", bufs=1, space="PSUM") as pscores_pool,
            tc.tile_pool(name="pqn", bufs=1, space="PSUM") as pqn_pool,
            tc.tile_pool(name="dram", bufs=2, space="DRAM") as dram,
        ):
            # ---------------- input DMAs (one queue, latency-ordered) ------
            wt8_sb = singles.tile([128, KO, C], FP8)
            nc.sync.dma_start(wt8_sb, wt8.rearrange("(ko p) o -> p ko o", p=128))
            x8_tiles = [singles.tile([128, KO, F], FP8, name=f"x8_{i}")
                        for i in range(NSLOTS)]
            x_bf_tiles = [singles.tile([128, KO, F], BF16, name=f"xbf{i}")
                          for i in range(NSLOTS)]

            def dma_x8(i):
                nc.sync.dma_start(
                    x8_tiles[i], xs8[i].rearrange("(ko p) f -> p ko f", p=128)
                )

            def dma_xbf(i):
                nc.sync.dma_start(
                    x_bf_tiles[i], xs[i].rearrange("(ko p) f -> p ko f", p=128)
                )

            for i in range(NSHOT_SLOTS):
                dma_x8(i)
            for i in range(NSHOT_SLOTS):
                dma_xbf(i)
            sw_sb = singles.tile([128, NSHOT_SLOTS, WAY], F32)
            nc.sync.dma_start(sw_sb, sw.rearrange("s p m -> p s m"))
            for i in range(NSHOT_SLOTS, NSLOTS):
                dma_x8(i)
                dma_xbf(i)

            # ---------------- constants ----------------
            onesC_f32 = singles.tile([128, 128], F32)
            nc.vector.memset(onesC_f32, 1.0)
            onesF_bf = singles.tile([128, 2], BF16)
            nc.vector.memset(onesF_bf, 1.0 / F)
            ones1 = singles.tile([128, 1], BF16)
            nc.vector.memset(ones1, 1.0)
            proto = singles.tile([128, KO, WAY], F32)
            nc.vector.memset(proto, 0.0)
            c_eps = singles.tile([128, 1], F32)
            nc.vector.memset(c_eps, 1e-30)
            c_ln10 = singles.tile([128, 1], F32)
            nc.vector.memset(c_ln10, LN10)
            pall = singles.tile([128, NPACKS, 2, 2, WAY], BF16)
            s_hat = singles.tile([128, KO, WAY], BF16)
            rsnb = singles.tile([128, WAY], F32)

            qn_ln = singles.tile([128, NQ_SLOTS, 2], BF16)
            psum_scores = pscores_pool.tile([128, NPACKS, 2, 2, WAY], F32)
            psum_qn = pqn_pool.tile([128, NQ_SLOTS, 2], F32)

            xm_tiles = [None] * NQ_SLOTS

            # ---------------- per-slot compute ----------------
            def conv_fp8(i):
                """fp8 DoubleRow 1x1 conv for slot i (psum = (16W) @ x)."""
                psum_t = pconv.tile([128, KO, F], F32, tag="conv", name=f"conv8_{i}")
                for oo in range(KO):
                    for h in range(2):
                        nc.tensor.matmul(
                            psum_t[:, oo, :],
                            wt8_sb[:, 2 * h : 2 * h + 2, 128 * oo : 128 * (oo + 1)],
                            x8_tiles[i][:, 2 * h : 2 * h + 2, :],
                            start=(h == 0),
                            stop=(h == 1),
                            perf_mode=DR,
                        )
                return psum_t

            def exp_mask(i, psum_t, sacc=None):
                """exp of the conv logits in one ACT op (scale undoes the x16
                on W); optionally accumulates sum_{ko,f} exp into sacc."""
                xe = xepool.tile([128, KO, F], BF16, tag="xe")
                kw = {}
                if sacc is not None:
                    kw["accum_out"] = sacc
                nc.scalar.activation(
                    xe, psum_t, EXP, scale=1.0 / W8_SCALE, **kw
                )
                return xe

            proto_bf = singles.tile([128, KO * WAY], BF16)

            def shot_reduction(i, xe, sacc, xm):
                """prototype contribution of shot slot i: per-channel masked
                sums (fused product+reduce) scaled by 1/S and the class
                one-hot, accumulated into proto."""
                with tc.high_priority():
                    nc.vector.tensor_mul(xm, x_bf_tiles[i], xe)
                    red = work.tile([128, KO, 1], F32, tag="red")
                    nc.vector.reduce_sum(red, xm, axis=mybir.AxisListType.X)
                    # softmax denominator S = sum_{c,f} exp(logit): sacc has
                    # the per-partition sums; the ones-matmul adds over
                    # partitions and broadcasts the total to all partitions
                    psum_s = pscratch.tile([128, F], F32, tag="scratch")
                    nc.tensor.matmul(
                        psum_s[:, :1], onesC_f32, sacc, start=True, stop=True
                    )
                    rS = work.tile([128, 1], F32, tag="rS")
                    nc.vector.reciprocal(rS, psum_s[:, :1])
                    w5b = work.tile([128, WAY], F32, tag="w5b")
                    nc.vector.tensor_scalar_mul(w5b, sw_sb[:, i, :], rS)
                    # contribution + accumulate on the idle Pool engine so
                    # the DVE queue stays clear for the next shot's sums
                    contrib = work.tile([128, KO, WAY], F32, tag="contrib")
                    nc.vector.tensor_tensor(
                        contrib,
                        red[:, :, 0][:, :, None].to_broadcast([128, KO, WAY]),
                        w5b[:, None, :].to_broadcast([128, KO, WAY]),
                        MULT,
                    )
                    if i < NSHOT_SLOTS - 1:
                        nc.vector.tensor_add(proto, proto, contrib)
                    else:
                        # final shot: emit the bf16 exchange payload directly
                        nc.vector.tensor_add(
                            proto_bf.rearrange("p (k w) -> p k w", k=KO),
                            proto,
                            contrib,
                        )

            def query_post(j, xm):
                """|q(f)|^2 directly in [f-part] orientation: stationary=xm2
                chunk, moving=ones -> out[f, 1]; then its log (phase 1)."""
                xm2 = xm2_pool.tile([128, KO, F], BF16, tag="xm2")
                nc.vector.tensor_mul(xm2, xm, xm)
                for h in range(2):
                    for k in range(KO):
                        nc.tensor.matmul(
                            psum_qn[:, j, h : h + 1],
                            xm2[:, k, 128 * h : 128 * (h + 1)],
                            ones1[:, :1],
                            start=(k == 0),
                            stop=(k == KO - 1),
                        )
                nc.scalar.activation(
                    qn_ln[:, j, :], psum_qn[:, j, :], LN, bias=c_eps
                )

            # ---------------- shot phase (slots 0..3) ----------------
            for i in range(NSHOT_SLOTS):
                psum_t = conv_fp8(i)
                sacc = work.tile([128, 1], F32, tag="sacc")
                xe = exp_mask(i, psum_t, sacc=sacc)
                if i == 3:
                    # shared slot: the masked product doubles as query j=0
                    xm = xmq_pool.tile([128, KO, F], BF16, tag="xmq")
                    xm_tiles[0] = xm
                else:
                    xm = xms_pool.tile([128, KO, F], BF16, tag="xms")
                shot_reduction(i, xe, sacc, xm)

            # ---------------- exchange prototype partials ----------------
            with tc.high_priority():
                ar_in = dram.tile([128, KO * WAY], BF16, tag="ar_in")
                ar_out = dram.tile([N_CORES, 128, KO * WAY], BF16, tag="ar_out")
                nc.sync.dma_start(ar_in, proto_bf)
                if kind == "skip":
                    for r in range(N_CORES):
                        nc.gpsimd.dma_start(ar_out[r], ar_in[:])
                else:
                    nc.gpsimd.collective_compute(
                        "AllGather",
                        mybir.AluOpType.bypass,
                        replica_groups=[list(range(N_CORES))],
                        ins=[ar_in[:].opt()],
                        outs=[ar_out[:].opt()],
                    )

            # slot 3's query-side phase-1 work (discarded on core 0)
            query_post(0, xm_tiles[0])

            # ---------------- queries phase 1 (slots 4..12) ----------------
            for j in range(1, NQ_SLOTS):
                i = 3 + j
                psum_t = conv_fp8(i)
                xe = exp_mask(i, psum_t)
                xm = xmq_pool.tile([128, KO, F], BF16, tag="xmq")
                xm_tiles[j] = xm
                nc.vector.tensor_mul(xm, x_bf_tiles[i], xe)
                query_post(j, xm)

            # ---------------- consume AllGather result ----------------
            with tc.high_priority():
                protoAll = singles.tile([128, N_CORES, KO * WAY], BF16)
                nc.sync.dma_start(protoAll, ar_out.rearrange("r p kw -> p r kw"))
                pa4 = work.tile([128, 4, KO * WAY], BF16, tag="pa4")
                nc.vector.tensor_add(pa4, protoAll[:, 0:4], protoAll[:, 4:8])
                pa2 = work.tile([128, 2, KO * WAY], BF16, tag="pa2")
                nc.vector.tensor_add(pa2, pa4[:, 0:2], pa4[:, 2:4])
                nc.vector.tensor_add(
                    s_hat.rearrange("p k w -> p (k w)"), pa2[:, 0], pa2[:, 1]
                )
            # 1/||proto|| (rsnb) is applied post-transpose, off the critical
            # path of the score matmuls.
            protosq = work.tile([128, KO, WAY], F32, tag="protosq")
            nc.vector.tensor_mul(protosq, s_hat, s_hat)
            psum_sn = pscratch.tile([128, F], F32, tag="scratch")
            for k in range(KO):
                nc.tensor.matmul(
                    psum_sn[:, :WAY],
                    onesC_f32,
                    protosq[:, k, :],
                    start=(k == 0),
                    stop=(k == KO - 1),
                )
            snln = work.tile([128, WAY], F32, tag="snln")
            nc.scalar.activation(snln, psum_sn[:, :WAY], LN, bias=c_eps)
            nc.scalar.activation(rsnb, snln, EXP, scale=-0.5)

            # ---------------- phase 2: scores + class softmax --------------
            def stage_a(pack):
                """scores already transposed: out[f, class] = xm8^T @ s_hat8"""
                for p2 in range(2):
                    j = 2 * pack + p2
                    for h in range(2):
                        for k in range(KO):
                            nc.tensor.matmul(
                                psum_scores[:, pack, p2, h, :],
                                xm_tiles[j][:, k, 128 * h : 128 * (h + 1)],
                                s_hat[:, k, :],
                                start=(k == 0),
                                stop=(k == KO - 1),
                            )

            def stage_b(pack):
                # rq = exp(-0.5*ln(|q|^2) + ln 10) = 10/|q|
                rq = work.tile([128, 2, 2], F32, tag="rq", name=f"rq{pack}")
                nc.scalar.activation(
                    rq, qn_ln[:, 2 * pack : 2 * pack + 2, :], EXP,
                    bias=c_ln10, scale=-0.5,
                )
                L = work.tile([128, 2, 2, WAY], BF16, tag="L", name=f"L{pack}")
                nc.vector.tensor_tensor(
                    L,
                    psum_scores[:, pack],
                    rq[:, :, :, None].to_broadcast([128, 2, 2, WAY]),
                    MULT,
                )
                LL = work.tile([128, 2, 2, WAY], BF16, tag="LL", name=f"LL{pack}")
                nc.vector.tensor_tensor(
                    LL,
                    L,
                    rsnb[:, None, None, :].to_broadcast([128, 2, 2, WAY]),
                    MULT,
                )
                E = work.tile([128, 2, 2, WAY], BF16, tag="E", name=f"E{pack}")
                nc.scalar.activation(E, LL, EXP)
                D = work.tile([128, 2, 2, 1], F32, tag="D", name=f"D{pack}")
                nc.vector.reduce_sum(D, E, axis=mybir.AxisListType.X)
                R = work.tile([128, 2, 2, 1], F32, tag="R", name=f"R{pack}")
                nc.vector.reciprocal(R, D)
                nc.vector.tensor_tensor(
                    pall[:, pack],
                    E,
                    R.to_broadcast([128, 2, 2, WAY]),
                    MULT,
                )

            for pack in range(NPACKS):
                stage_a(pack)
            for pack in range(NPACKS):
                stage_b(pack)

            # batched partition-sum over all packs, folding the two spatial
            # halves by accumulating both into the same psum rows
            psO = pscratch.tile([128, F], F32, tag="scratch", name="psO")
            for h in range(2):
                nc.tensor.matmul(
                    psO[:2, : NPACKS * 2 * WAY],
                    onesF_bf,
                    pall[:, :, :, h, :],
                    start=(h == 0),
                    stop=(h == 1),
                )
            out_sb = work.tile([1, NQ_SLOTS * WAY], F32, tag="po_sb")
            nc.any.tensor_copy(out_sb, psO[:1, : NQ_SLOTS * WAY])
            nc.sync.dma_start(out[:], out_sb[0:1, :])

    nc.finalize()
    return nc


_NC_CACHE = {}


def _get_nc():
    if "nc" not in _NC_CACHE:
        _NC_CACHE["nc"] = build_nc()
    return _NC_CACHE["nc"]


SHOTS_PER_CORE = [4, 3, 3, 3, 3, 3, 3, 3]       # sums to 25
QUERIES_PER_CORE = [9, 10, 10, 10, 9, 9, 9, 9]  # sums to 75


def _assignments():
    """Per-core (shot global ids, query global ids)."""
    shots = [20 * c + j for c in range(WAY) for j in range(SHOT)]
    queries = [20 * c + SHOT + j for c in range(WAY) for j in range(15)]
    so = np.cumsum([0] + SHOTS_PER_CORE)
    qo = np.cumsum([0] + QUERIES_PER_CORE)
    return [
        (shots[so[k] : so[k + 1]], queries[qo[k] : qo[k + 1]]) for k in range(N_CORES)
    ]


def _core_slot_layout(k):
    """core k: shots fill slots 0..n_s-1, queries fill slots q_start..
    (core 0's 4th shot occupies slot 3; its query j=0 is unused)."""
    n_s = SHOTS_PER_CORE[k]
    q_start = max(n_s, 3)
    return n_s, q_start


def _make_in_maps(x, W, b):
    assert np.all(b == 0.0), "kernel folds b==0 (spec: bias is zeros)"
    wtT = np.ascontiguousarray(W.T)
    wt8 = (wtT * W8_SCALE).astype(ml_dtypes.float8_e4m3)
    x_bf = x.astype(ml_dtypes.bfloat16)
    x_f8 = x.astype(ml_dtypes.float8_e4m3)
    assign = _assignments()
    in_maps = []
    for k in range(N_CORES):
        s_list, q_list = assign[k]
        n_s, q_start = _core_slot_layout(k)
        xs_core = np.zeros((NSLOTS, C, F), dtype=ml_dtypes.bfloat16)
        xs_core[:n_s] = x_bf[s_list]
        xs_core[q_start : q_start + len(q_list)] = x_bf[q_list]
        xs8_core = np.zeros((NSLOTS, C, F), dtype=ml_dtypes.float8_e4m3)
        xs8_core[:n_s] = x_f8[s_list]
        xs8_core[q_start : q_start + len(q_list)] = x_f8[q_list]
        sw_core = np.zeros((NSHOT_SLOTS, WAY), dtype=np.float32)
        for slot, g in enumerate(s_list):
            sw_core[slot, g // 20] = 1.0
        sw_b = np.broadcast_to(
            sw_core[:, None, :], (NSHOT_SLOTS, 128, WAY)
        ).astype(np.float32)
        in_maps.append(
            {
                "xs": xs_core,
                "xs8": xs8_core,
                "wt8": wt8,
                "sw": np.ascontiguousarray(sw_b),
            }
        )
    return in_maps


def kernel(x, W, b):
    x = np.asarray(x, dtype=np.float32).reshape(100, C, F)
    W = np.asarray(W, dtype=np.float32)
    b = np.asarray(b, dtype=np.float32)

    nc = _get_nc()
    in_maps = _make_in_maps(x, W, b)
    res = run_bass_kernel_spmd(nc, in_maps, core_ids=list(range(N_CORES)))

    assign = _assignments()
    final = np.zeros((75, WAY), dtype=np.float32)
    for k in range(N_CORES):
        out_core = np.asarray(res.results[k]["out"], dtype=np.float32).reshape(
            NQ_SLOTS, WAY
        )
        _, q_list = assign[k]
        n_s, q_start = _core_slot_layout(k)
        for slot, g in enumerate(q_list):
            c, j = divmod(g, 20)
            final[15 * c + (j - SHOT)] = out_core[q_start - 3 + slot]
    return final


# revision 40
# speedup vs baseline: 1.0199x; 1.0199x over previous
"""Trainium2 Bass kernel for the few-shot knn-attention module.

Pipeline per sample (512 ch, 16x16 spatial):
  mask = softmax_{c,h,w}(W @ x); xm = x * mask  (mask kept unnormalized on
  device; the softmax denominator is folded algebraically; b==0 by spec)
  prototypes s = mean over 5 shots+space of xm; queries scored by cosine
  similarity against s; softmax over classes; mean over space -> (75, 5).

Distribution: data-parallel over the 100 samples on 8 NeuronCores with 13
slots/core (3 shot slots + 1 shared shot/query slot + 9 query slots,
zero-padded).  All 1x1 convs run in fp8 DoubleRow (W pre-scaled by 16,
folded back in the exp); x stays bf16 for the mask product so the
cosine path keeps full precision.  The prototype partial sums are
exchanged with an AllGather + local tree-sum (modeled ~1.8x cheaper
than AllReduce); the score matmuls are emitted "pre-transposed"
(stationary=xm chunk, moving=prototypes) so scores and |q|^2 land in
[spatial-position, class] orientation directly - no PSUM->SBUF copies
or PE transposes in the tail.
"""

import numpy as np
import ml_dtypes

import concourse.bass as bass
import concourse.mybir as mybir
import concourse.tile as tile
from concourse import bacc
from concourse.bass_utils import run_bass_kernel_spmd

# Force the act-table chooser onto the one set containing BOTH Exp and Ln
# ("natural_log_exp_and_others") so the kernel pays a single table load
# instead of swapping between the exp-only and ln-only sets (~2.7us each).
import concourse.hw_specs as _hw_specs

_ORIG_GET_ACT_TABLES = _hw_specs.get_activation_tables


def _nl_exp_only_tables(arch):
    t = _ORIG_GET_ACT_TABLES(arch)
    return {
        k: (v if k == "natural_log_exp_and_others" else set()) for k, v in t.items()
    }


bacc.get_activation_tables = _nl_exp_only_tables

N_CORES = 8
WAY = 5
SHOT = 5
C = 512
F = 256  # 16*16
KO = C // 128  # 4 partition tiles of the channel dim
NSHOT_SLOTS = 4   # slots 0..3 feed the prototype partials (slot 3 shared)
NQ_SLOTS = 10     # slots 3..12 are query-capable (slot 3 shared)
NSLOTS = 13
NPACKS = NQ_SLOTS // 2
W8_SCALE = 16.0   # W is pre-scaled by 16 for fp8; folded back in the exp
LN10 = float(np.log(10.0))

F32 = mybir.dt.float32
BF16 = mybir.dt.bfloat16
FP8 = mybir.dt.float8e4
EXP = mybir.ActivationFunctionType.Exp
LN = mybir.ActivationFunctionType.Ln
DR = mybir.MatmulPerfMode.DoubleRow
MULT = mybir.AluOpType.mult
ADD = mybir.AluOpType.add


def build_nc(kind="AllGather"):
    nc = bacc.Bacc(None, target_bir_lowering=False)
    xs = nc.dram_tensor("xs", [NSLOTS, C, F], BF16, kind="ExternalInput")
    xs8 = nc.dram_tensor("xs8", [NSLOTS, C, F], FP8, kind="ExternalInput")
    wt8 = nc.dram_tensor("wt8", [C, C], FP8, kind="ExternalInput")
    sw = nc.dram_tensor("sw", [NSHOT_SLOTS, 128, WAY], F32, kind="ExternalInput")
    out = nc.dram_tensor("out", [1, NQ_SLOTS * WAY], F32, kind="ExternalOutput")

    with tile.TileContext(nc) as tc:
        with (
            tc.tile_pool(name="singles", bufs=1) as singles,
            tc.tile_pool(name="xepool", bufs=3) as xepool,
            tc.tile_pool(name="xmq", bufs=NQ_SLOTS) as xmq_pool,
            tc.tile_pool(name="xms", bufs=2) as xms_pool,
            tc.tile_pool(name="xm2", bufs=3) as xm2_pool,
            tc.tile_pool(name="work", bufs=4) as work,
            tc.tile_pool(name="pconv", bufs=2, space="PSUM") as pconv,
            tc.tile_pool(name="pscratch", bufs=2, space="PSUM") as pscratch,
            tc.tile_pool(name="pscores", bufs=1, space="PSUM") as pscores_pool,
            tc.tile_pool(name="pqn", bufs=1, space="PSUM") as pqn_pool,
            tc.tile_pool(name="dram", bufs=2, space="DRAM") as dram,
        ):
            # ---------------- input DMAs (one queue, latency-ordered) ------
            wt8_sb = singles.tile([128, KO, C], FP8)
            nc.sync.dma_start(wt8_sb, wt8.rearrange("(ko p) o -> p ko o", p=128))
            x8_tiles = [singles.tile([128, KO, F], FP8, name=f"x8_{i}")
                        for i in range(NSLOTS)]
            x_bf_tiles = [singles.tile([128, KO, F], BF16, name=f"xbf{i}")
                          for i in range(NSLOTS)]

            def dma_x8(i):
                nc.sync.dma_start(
                    x8_tiles[i], xs8[i].rearrange("(ko p) f -> p ko f", p=128)
                )

            def dma_xbf(i):
                nc.sync.dma_start(
                    x_bf_tiles[i], xs[i].rearrange("(ko p) f -> p ko f", p=128)
                )

            for i in range(NSHOT_SLOTS):
                dma_x8(i)
            for i in range(NSHOT_SLOTS):
                dma_xbf(i)
            sw_sb = singles.tile([128, NSHOT_SLOTS, WAY], F32)
            nc.sync.dma_start(sw_sb, sw.rearrange("s p m -> p s m"))
            for i in range(NSHOT_SLOTS, NSLOTS):
                dma_x8(i)
                dma_xbf(i)

            # ---------------- constants ----------------
            onesC_f32 = singles.tile([128, 128], F32)
            nc.vector.memset(onesC_f32, 1.0)
            onesF_bf = singles.tile([128, 2], BF16)
            nc.vector.memset(onesF_bf, 1.0 / F)
            ones1 = singles.tile([128, 1], BF16)
            nc.vector.memset(ones1, 1.0)
            proto = singles.tile([128, KO, WAY], F32)
            nc.vector.memset(proto, 0.0)
            c_eps = singles.tile([128, 1], F32)
            nc.vector.memset(c_eps, 1e-30)
            c_ln10 = singles.tile([128, 1], F32)
            nc.vector.memset(c_ln10, LN10)
            pall = singles.tile([128, NPACKS, 2, 2, WAY], BF16)
            s_hat = singles.tile([128, KO, WAY], BF16)
            rsnb = singles.tile([128, WAY], F32)

            qn_ln = singles.tile([128, NQ_SLOTS, 2], BF16)
            psum_scores = pscores_pool.tile([128, NPACKS, 2, 2, WAY], F32)
            psum_qn = pqn_pool.tile([128, NQ_SLOTS, 2], F32)

            xm_tiles = [None] * NQ_SLOTS

            # ---------------- per-slot compute ----------------
            def conv_fp8(i):
                """fp8 DoubleRow 1x1 conv for slot i (psum = (16W) @ x)."""
                psum_t = pconv.tile([128, KO, F], F32, tag="conv", name=f"conv8_{i}")
                for oo in range(KO):
                    for h in range(2):
                        nc.tensor.matmul(
                            psum_t[:, oo, :],
                            wt8_sb[:, 2 * h : 2 * h + 2, 128 * oo : 128 * (oo + 1)],
                            x8_tiles[i][:, 2 * h : 2 * h + 2, :],
                            start=(h == 0),
                            stop=(h == 1),
                            perf_mode=DR,
                        )
                return psum_t

            def exp_mask(i, psum_t, sacc=None):
                """exp of the conv logits in one ACT op (scale undoes the x16
                on W); optionally accumulates sum_{ko,f} exp into sacc."""
                xe = xepool.tile([128, KO, F], BF16, tag="xe")
                kw = {}
                if sacc is not None:
                    kw["accum_out"] = sacc
                nc.scalar.activation(
                    xe, psum_t, EXP, scale=1.0 / W8_SCALE, **kw
                )
                return xe

            proto_bf = singles.tile([128, KO * WAY], BF16)

            def shot_reduction(i, xe, sacc, xm):
                """prototype contribution of shot slot i: per-channel masked
                sums (fused product+reduce) scaled by 1/S and the class
                one-hot, accumulated into proto."""
                with tc.high_priority():
                    nc.vector.tensor_mul(xm, x_bf_tiles[i], xe)
                    red = work.tile([128, KO, 1], F32, tag="red")
                    nc.vector.reduce_sum(red, xm, axis=mybir.AxisListType.X)
                    # softmax denominator S = sum_{c,f} exp(logit): sacc has
                    # the per-partition sums; the ones-matmul adds over
                    # partitions and broadcasts the total to all partitions
                    psum_s = pscratch.tile([128, F], F32, tag="scratch")
                    nc.tensor.matmul(
                        psum_s[:, :1], onesC_f32, sacc, start=True, stop=True
                    )
                    rS = work.tile([128, 1], F32, tag="rS")
                    nc.vector.reciprocal(rS, psum_s[:, :1])
                    w5b = work.tile([128, WAY], F32, tag="w5b")
                    nc.vector.tensor_scalar_mul(w5b, sw_sb[:, i, :], rS)
                    # contribution + accumulate on the idle Pool engine so
                    # the DVE queue stays clear for the next shot's sums
                    contrib = work.tile([128, KO, WAY], F32, tag="contrib")
                    nc.vector.tensor_tensor(
                        contrib,
                        red[:, :, 0][:, :, None].to_broadcast([128, KO, WAY]),
                        w5b[:, None, :].to_broadcast([128, KO, WAY]),
                        MULT,
                    )
                    if i < NSHOT_SLOTS - 1:
                        nc.vector.tensor_add(proto, proto, contrib)
                    else:
                        # final shot: emit the bf16 exchange payload directly
                        nc.vector.tensor_add(
                            proto_bf.rearrange("p (k w) -> p k w", k=KO),
                            proto,
                            contrib,
                        )

            def query_post(j, xm):
                """|q(f)|^2 directly in [f-part] orientation: stationary=xm2
                chunk, moving=ones -> out[f, 1]; then its log (phase 1)."""
                xm2 = xm2_pool.tile([128, KO, F], BF16, tag="xm2")
                nc.vector.tensor_mul(xm2, xm, xm)
                for h in range(2):
                    for k in range(KO):
                        nc.tensor.matmul(
                            psum_qn[:, j, h : h + 1],
                            xm2[:, k, 128 * h : 128 * (h + 1)],
                            ones1[:, :1],
                            start=(k == 0),
                            stop=(k == KO - 1),
                        )
                nc.scalar.activation(
                    qn_ln[:, j, :], psum_qn[:, j, :], LN, bias=c_eps
                )

            # ---------------- shot phase (slots 0..3) ----------------
            for i in range(NSHOT_SLOTS):
                psum_t = conv_fp8(i)
                sacc = work.tile([128, 1], F32, tag="sacc")
                xe = exp_mask(i, psum_t, sacc=sacc)
                if i == 3:
                    # shared slot: the masked product doubles as query j=0
                    xm = xmq_pool.tile([128, KO, F], BF16, tag="xmq")
                    xm_tiles[0] = xm
                else:
                    xm = xms_pool.tile([128, KO, F], BF16, tag="xms")
                shot_reduction(i, xe, sacc, xm)

            # ---------------- exchange prototype partials ----------------
            with tc.high_priority():
                ar_in = dram.tile([128, KO * WAY], BF16, tag="ar_in")
                ar_out = dram.tile([N_CORES, 128, KO * WAY], BF16, tag="ar_out")
                nc.sync.dma_start(ar_in, proto_bf)
                if kind == "skip":
                    for r in range(N_CORES):
                        nc.gpsimd.dma_start(ar_out[r], ar_in[:])
                else:
                    nc.gpsimd.collective_compute(
                        "AllGather",
                        mybir.AluOpType.bypass,
                        replica_groups=[list(range(N_CORES))],
                        ins=[ar_in[:].opt()],
                        outs=[ar_out[:].opt()],
                    )

            # slot 3's query-side phase-1 work (discarded on core 0)
            query_post(0, xm_tiles[0])

            # ---------------- queries phase 1 (slots 4..12) ----------------
            for j in range(1, NQ_SLOTS):
                i = 3 + j
                psum_t = conv_fp8(i)
                xe = exp_mask(i, psum_t)
                xm = xmq_pool.tile([128, KO, F], BF16, tag="xmq")
                xm_tiles[j] = xm
                nc.vector.tensor_mul(xm, x_bf_tiles[i], xe)
                query_post(j, xm)

            # rq = exp(-0.5*ln(|q|^2) + ln 10) = 10/|q| for all queries —
            # computed during the collective window (only needs phase-1 data)
            rq = work.tile([128, NQ_SLOTS, 2], F32, tag="rq")
            nc.scalar.activation(rq, qn_ln, EXP, bias=c_ln10, scale=-0.5)

            # ---------------- consume AllGather result ----------------
            with tc.high_priority():
                protoAll = singles.tile([128, N_CORES, KO * WAY], BF16)
                half = N_CORES // 2
                nc.sync.dma_start(
                    protoAll[:, :half], ar_out[:half].rearrange("r p kw -> p r kw")
                )
                nc.scalar.dma_start(
                    protoAll[:, half:], ar_out[half:].rearrange("r p kw -> p r kw")
                )
                pa4 = work.tile([128, 4, KO * WAY], BF16, tag="pa4")
                nc.vector.tensor_add(pa4, protoAll[:, 0:4], protoAll[:, 4:8])
                pa2 = work.tile([128, 2, KO * WAY], BF16, tag="pa2")
                nc.vector.tensor_add(pa2, pa4[:, 0:2], pa4[:, 2:4])
                nc.vector.tensor_add(
                    s_hat.rearrange("p k w -> p (k w)"), pa2[:, 0], pa2[:, 1]
                )
            # 1/||proto|| (rsnb) is applied post-transpose, off the critical
            # path of the score matmuls.
            protosq = work.tile([128, KO, WAY], F32, tag="protosq")
            nc.vector.tensor_mul(protosq, s_hat, s_hat)
            psum_sn = pscratch.tile([128, F], F32, tag="scratch")
            for k in range(KO):
                nc.tensor.matmul(
                    psum_sn[:, :WAY],
                    onesC_f32,
                    protosq[:, k, :],
                    start=(k == 0),
                    stop=(k == KO - 1),
                )
            snln = work.tile([128, WAY], F32, tag="snln")
            nc.scalar.activation(snln, psum_sn[:, :WAY], LN, bias=c_eps)
            nc.scalar.activation(rsnb, snln, EXP, scale=-0.5)

            # ---------------- phase 2: scores + class softmax --------------
            def stage_a(pack):
                """scores already transposed: out[f, class] = xm8^T @ s_hat8"""
                for p2 in range(2):
                    j = 2 * pack + p2
                    for h in range(2):
                        for k in range(KO):
                            nc.tensor.matmul(
                                psum_scores[:, pack, p2, h, :],
                                xm_tiles[j][:, k, 128 * h : 128 * (h + 1)],
                                s_hat[:, k, :],
                                start=(k == 0),
                                stop=(k == KO - 1),
                            )

            for pack in range(NPACKS):
                stage_a(pack)

            # ---------------- batched class softmax over all packs ---------
            # 3 free dims max per ISA op: flatten (pack, p2, h) -> 20 rows
            NR = NPACKS * 4
            L = work.tile([128, NR, WAY], BF16, tag="L")
            nc.vector.tensor_tensor(
                L,
                psum_scores.rearrange("p a q h m -> p (a q h) m"),
                rq.rearrange("p q h -> p (q h)")[:, :, None].to_broadcast(
                    [128, NR, WAY]
                ),
                MULT,
            )
            LL = work.tile([128, NR, WAY], BF16, tag="LL")
            nc.vector.tensor_tensor(
                LL,
                L,
                rsnb[:, None, :].to_broadcast([128, NR, WAY]),
                MULT,
            )
            E = work.tile([128, NR, WAY], BF16, tag="E")
            nc.scalar.activation(E, LL, EXP)
            D = work.tile([128, NR, 1], F32, tag="D")
            nc.vector.reduce_sum(D, E, axis=mybir.AxisListType.X)
            R = work.tile([128, NR, 1], F32, tag="R")
            nc.vector.reciprocal(R, D)
            nc.vector.tensor_tensor(
                pall.rearrange("p a q h m -> p (a q h) m"),
                E,
                R.to_broadcast([128, NR, WAY]),
                MULT,
            )

            # batched partition-sum over all packs, folding the two spatial
            # halves by accumulating both into the same psum rows
            psO = pscratch.tile([128, F], F32, tag="scratch", name="psO")
            for h in range(2):
                nc.tensor.matmul(
                    psO[:2, : NPACKS * 2 * WAY],
                    onesF_bf,
                    pall[:, :, :, h, :],
                    start=(h == 0),
                    stop=(h == 1),
                )
            out_sb = work.tile([1, NQ_SLOTS * WAY], F32, tag="po_sb")
            nc.any.tensor_copy(out_sb, psO[:1, : NQ_SLOTS * WAY])
            nc.sync.dma_start(out[:], out_sb[0:1, :])

    nc.finalize()
    return nc


_NC_CACHE = {}


def _get_nc():
    if "nc" not in _NC_CACHE:
        _NC_CACHE["nc"] = build_nc()
    return _NC_CACHE["nc"]


SHOTS_PER_CORE = [4, 3, 3, 3, 3, 3, 3, 3]       # sums to 25
QUERIES_PER_CORE = [9, 10, 10, 10, 9, 9, 9, 9]  # sums to 75


def _assignments():
    """Per-core (shot global ids, query global ids)."""
    shots = [20 * c + j for c in range(WAY) for j in range(SHOT)]
    queries = [20 * c + SHOT + j for c in range(WAY) for j in range(15)]
    so = np.cumsum([0] + SHOTS_PER_CORE)
    qo = np.cumsum([0] + QUERIES_PER_CORE)
    return [
        (shots[so[k] : so[k + 1]], queries[qo[k] : qo[k + 1]]) for k in range(N_CORES)
    ]


def _core_slot_layout(k):
    """core k: shots fill slots 0..n_s-1, queries fill slots q_start..
    (core 0's 4th shot occupies slot 3; its query j=0 is unused)."""
    n_s = SHOTS_PER_CORE[k]
    q_start = max(n_s, 3)
    return n_s, q_start


def _make_in_maps(x, W, b):
    assert np.all(b == 0.0), "kernel folds b==0 (spec: bias is zeros)"
    wtT = np.ascontiguousarray(W.T)
    wt8 = (wtT * W8_SCALE).astype(ml_dtypes.float8_e4m3)
    x_bf = x.astype(ml_dtypes.bfloat16)
    x_f8 = x.astype(ml_dtypes.float8_e4m3)
    assign = _assignments()
    in_maps = []
    for k in range(N_CORES):
        s_list, q_list = assign[k]
        n_s, q_start = _core_slot_layout(k)
        xs_core = np.zeros((NSLOTS, C, F), dtype=ml_dtypes.bfloat16)
        xs_core[:n_s] = x_bf[s_list]
        xs_core[q_start : q_start + len(q_list)] = x_bf[q_list]
        xs8_core = np.zeros((NSLOTS, C, F), dtype=ml_dtypes.float8_e4m3)
        xs8_core[:n_s] = x_f8[s_list]
        xs8_core[q_start : q_start + len(q_list)] = x_f8[q_list]
        sw_core = np.zeros((NSHOT_SLOTS, WAY), dtype=np.float32)
        for slot, g in enumerate(s_list):
            sw_core[slot, g // 20] = 1.0
        sw_b = np.broadcast_to(
            sw_core[:, None, :], (NSHOT_SLOTS, 128, WAY)
        ).astype(np.float32)
        in_maps.append(
            {
                "xs": xs_core,
                "xs8": xs8_core,
                "wt8": wt8,
                "sw": np.ascontiguousarray(sw_b),
            }
        )
    return in_maps


def kernel(x, W, b):
    x = np.asarray(x, dtype=np.float32).reshape(100, C, F)
    W = np.asarray(W, dtype=np.float32)
    b = np.asarray(b, dtype=np.float32)

    nc = _get_nc()
    in_maps = _make_in_maps(x, W, b)
    res = run_bass_kernel_spmd(nc, in_maps, core_ids=list(range(N_CORES)))

    assign = _assignments()
    final = np.zeros((75, WAY), dtype=np.float32)
    for k in range(N_CORES):
        out_core = np.asarray(res.results[k]["out"], dtype=np.float32).reshape(
            NQ_SLOTS, WAY
        )
        _, q_list = assign[k]
        n_s, q_start = _core_slot_layout(k)
        for slot, g in enumerate(q_list):
            c, j = divmod(g, 20)
            final[15 * c + (j - SHOT)] = out_core[q_start - 3 + slot]
    return final


# revision 44
# speedup vs baseline: 1.0309x; 1.0108x over previous
"""Trainium2 Bass kernel for the few-shot knn-attention module.

Pipeline per sample (512 ch, 16x16 spatial):
  mask = softmax_{c,h,w}(W @ x); xm = x * mask  (mask kept unnormalized on
  device; the softmax denominator is folded algebraically; b==0 by spec)
  prototypes s = mean over 5 shots+space of xm; queries scored by cosine
  similarity against s; softmax over classes; mean over space -> (75, 5).

Distribution: data-parallel over the 100 samples on 8 NeuronCores with 13
slots/core (3 shot slots + 1 shared shot/query slot + 9 query slots,
zero-padded).  All 1x1 convs run in fp8 DoubleRow (W pre-scaled by 16,
folded back in the exp); x stays bf16 for the mask product so the
cosine path keeps full precision.  The prototype partial sums are
exchanged with an AllGather + local tree-sum (modeled ~1.8x cheaper
than AllReduce); the score matmuls are emitted "pre-transposed"
(stationary=xm chunk, moving=prototypes) so scores and |q|^2 land in
[spatial-position, class] orientation directly - no PSUM->SBUF copies
or PE transposes in the tail.
"""

import numpy as np
import ml_dtypes

import concourse.bass as bass
import concourse.mybir as mybir
import concourse.tile as tile
from concourse import bacc
from concourse.bass_utils import run_bass_kernel_spmd

# Force the act-table chooser onto the one set containing BOTH Exp and Ln
# ("natural_log_exp_and_others") so the kernel pays a single table load
# instead of swapping between the exp-only and ln-only sets (~2.7us each).
import concourse.hw_specs as _hw_specs

_ORIG_GET_ACT_TABLES = _hw_specs.get_activation_tables


def _nl_exp_only_tables(arch):
    t = _ORIG_GET_ACT_TABLES(arch)
    return {
        k: (v if k == "natural_log_exp_and_others" else set()) for k, v in t.items()
    }


bacc.get_activation_tables = _nl_exp_only_tables

N_CORES = 8
WAY = 5
SHOT = 5
C = 512
F = 256  # 16*16
KO = C // 128  # 4 partition tiles of the channel dim
NSHOT_SLOTS = 4   # slots 0..3 feed the prototype partials (slot 3 shared)
NQ_SLOTS = 10     # slots 3..12 are query-capable (slot 3 shared)
NSLOTS = 13
NPACKS = NQ_SLOTS // 2
W8_SCALE = 16.0   # W is pre-scaled by 16 for fp8; folded back in the exp
LN10 = float(np.log(10.0))
QMUL_WAIT_MS = 0.0125  # clock-pin query DVE work past the prototype chain

F32 = mybir.dt.float32
BF16 = mybir.dt.bfloat16
FP8 = mybir.dt.float8e4
EXP = mybir.ActivationFunctionType.Exp
LN = mybir.ActivationFunctionType.Ln
DR = mybir.MatmulPerfMode.DoubleRow
MULT = mybir.AluOpType.mult
ADD = mybir.AluOpType.add


def build_nc(kind="AllGather"):
    nc = bacc.Bacc(None, target_bir_lowering=False)
    xs = nc.dram_tensor("xs", [NSLOTS, C, F], BF16, kind="ExternalInput")
    xs8 = nc.dram_tensor("xs8", [NSLOTS, C, F], FP8, kind="ExternalInput")
    wt8 = nc.dram_tensor("wt8", [C, C], FP8, kind="ExternalInput")
    sw = nc.dram_tensor("sw", [NSHOT_SLOTS, 128, WAY], F32, kind="ExternalInput")
    out = nc.dram_tensor("out", [1, NQ_SLOTS * WAY], F32, kind="ExternalOutput")

    with tile.TileContext(nc) as tc:
        with (
            tc.tile_pool(name="singles", bufs=1) as singles,
            tc.tile_pool(name="xepool", bufs=3) as xepool,
            tc.tile_pool(name="xmq", bufs=NQ_SLOTS) as xmq_pool,
            tc.tile_pool(name="xms", bufs=2) as xms_pool,
            tc.tile_pool(name="xm2", bufs=3) as xm2_pool,
            tc.tile_pool(name="work", bufs=4) as work,
            tc.tile_pool(name="pconv", bufs=2, space="PSUM") as pconv,
            tc.tile_pool(name="pscratch", bufs=2, space="PSUM") as pscratch,
            tc.tile_pool(name="pscores", bufs=1, space="PSUM") as pscores_pool,
            tc.tile_pool(name="pqn", bufs=1, space="PSUM") as pqn_pool,
            tc.tile_pool(name="dram", bufs=2, space="DRAM") as dram,
        ):
            # ---------------- input DMAs (one queue, latency-ordered) ------
            wt8_sb = singles.tile([128, KO, C], FP8)
            x8_tiles = [singles.tile([128, KO, F], FP8, name=f"x8_{i}")
                        for i in range(NSLOTS)]
            x_bf_tiles = [singles.tile([128, KO, F], BF16, name=f"xbf{i}")
                          for i in range(NSLOTS)]

            def dma_x8(i):
                nc.sync.dma_start(
                    x8_tiles[i], xs8[i].rearrange("(ko p) f -> p ko f", p=128)
                )

            def dma_xbf(i):
                nc.sync.dma_start(
                    x_bf_tiles[i], xs[i].rearrange("(ko p) f -> p ko f", p=128)
                )

            dma_x8(0)
            nc.sync.dma_start(wt8_sb, wt8.rearrange("(ko p) o -> p ko o", p=128))
            for i in range(1, NSHOT_SLOTS):
                dma_x8(i)
            for i in range(NSHOT_SLOTS):
                dma_xbf(i)
            sw_sb = singles.tile([128, NSHOT_SLOTS, WAY], F32)
            nc.sync.dma_start(sw_sb, sw.rearrange("s p m -> p s m"))
            for i in range(NSHOT_SLOTS, NSLOTS):
                dma_x8(i)
                dma_xbf(i)

            # ---------------- constants ----------------
            onesC_f32 = singles.tile([128, 128], F32)
            nc.vector.memset(onesC_f32, 1.0)
            onesF_bf = singles.tile([128, 2], BF16)
            nc.vector.memset(onesF_bf, 1.0 / F)
            ones1 = singles.tile([128, 1], BF16)
            nc.vector.memset(ones1, 1.0)
            proto = singles.tile([128, KO, WAY], F32)
            nc.vector.memset(proto, 0.0)
            c_eps = singles.tile([128, 1], F32)
            nc.vector.memset(c_eps, 1e-30)
            c_ln10 = singles.tile([128, 1], F32)
            nc.vector.memset(c_ln10, LN10)
            pall = singles.tile([128, NPACKS, 2, 2, WAY], BF16)
            s_hat = singles.tile([128, KO, WAY], BF16)
            rsnb = singles.tile([128, WAY], F32)

            qn_ln = singles.tile([128, NQ_SLOTS, 2], BF16)
            psum_scores = pscores_pool.tile([128, NPACKS, 2, 2, WAY], F32)
            psum_qn = pqn_pool.tile([128, NQ_SLOTS, 2], F32)

            xm_tiles = [None] * NQ_SLOTS

            # ---------------- per-slot compute ----------------
            def conv_fp8(i):
                """fp8 DoubleRow 1x1 conv for slot i (psum = (16W) @ x)."""
                psum_t = pconv.tile([128, KO, F], F32, tag="conv", name=f"conv8_{i}")
                for oo in range(KO):
                    for h in range(2):
                        nc.tensor.matmul(
                            psum_t[:, oo, :],
                            wt8_sb[:, 2 * h : 2 * h + 2, 128 * oo : 128 * (oo + 1)],
                            x8_tiles[i][:, 2 * h : 2 * h + 2, :],
                            start=(h == 0),
                            stop=(h == 1),
                            perf_mode=DR,
                        )
                return psum_t

            def exp_mask(i, psum_t, sacc=None):
                """exp of the conv logits in one ACT op (scale undoes the x16
                on W); optionally accumulates sum_{ko,f} exp into sacc."""
                xe = xepool.tile([128, KO, F], BF16, tag="xe")
                kw = {}
                if sacc is not None:
                    kw["accum_out"] = sacc
                nc.scalar.activation(
                    xe, psum_t, EXP, scale=1.0 / W8_SCALE, **kw
                )
                return xe

            proto_bf = singles.tile([128, KO * WAY], BF16)

            def shot_reduction(i, xe, sacc, xm):
                """prototype contribution of shot slot i: per-channel masked
                sums (fused product+reduce) scaled by 1/S and the class
                one-hot, accumulated into proto."""
                with tc.high_priority():
                    nc.vector.tensor_mul(xm, x_bf_tiles[i], xe)
                    red = work.tile([128, KO, 1], F32, tag="red")
                    nc.vector.reduce_sum(red, xm, axis=mybir.AxisListType.X)
                    # softmax denominator S = sum_{c,f} exp(logit): sacc has
                    # the per-partition sums; the ones-matmul adds over
                    # partitions and broadcasts the total to all partitions
                    psum_s = pscratch.tile([128, F], F32, tag="scratch")
                    nc.tensor.matmul(
                        psum_s[:, :1], onesC_f32, sacc, start=True, stop=True
                    )
                    rS = work.tile([128, 1], F32, tag="rS")
                    nc.vector.reciprocal(rS, psum_s[:, :1])
                    w5b = work.tile([128, WAY], F32, tag="w5b")
                    nc.vector.tensor_scalar_mul(w5b, sw_sb[:, i, :], rS)
                    # contribution + accumulate on the idle Pool engine so
                    # the DVE queue stays clear for the next shot's sums
                    contrib = work.tile([128, KO, WAY], F32, tag="contrib")
                    nc.vector.tensor_tensor(
                        contrib,
                        red[:, :, 0][:, :, None].to_broadcast([128, KO, WAY]),
                        w5b[:, None, :].to_broadcast([128, KO, WAY]),
                        MULT,
                    )
                    if i < NSHOT_SLOTS - 1:
                        nc.vector.tensor_add(proto, proto, contrib)
                    else:
                        # final shot: emit the bf16 exchange payload directly
                        nc.vector.tensor_add(
                            proto_bf.rearrange("p (k w) -> p k w", k=KO),
                            proto,
                            contrib,
                        )

            def query_post(j, xm):
                """|q(f)|^2 directly in [f-part] orientation: stationary=xm2
                chunk, moving=ones -> out[f, 1]; then its log (phase 1)."""
                xm2 = xm2_pool.tile([128, KO, F], BF16, tag="xm2")
                nc.vector.tensor_mul(xm2, xm, xm)
                for h in range(2):
                    for k in range(KO):
                        nc.tensor.matmul(
                            psum_qn[:, j, h : h + 1],
                            xm2[:, k, 128 * h : 128 * (h + 1)],
                            ones1[:, :1],
                            start=(k == 0),
                            stop=(k == KO - 1),
                        )
                nc.scalar.activation(
                    qn_ln[:, j, :], psum_qn[:, j, :], LN, bias=c_eps
                )

            # ---------------- shot phase (slots 0..3) ----------------
            for i in range(NSHOT_SLOTS):
                psum_t = conv_fp8(i)
                sacc = work.tile([128, 1], F32, tag="sacc")
                xe = exp_mask(i, psum_t, sacc=sacc)
                if i == 3:
                    # shared slot: the masked product doubles as query j=0
                    xm = xmq_pool.tile([128, KO, F], BF16, tag="xmq")
                    xm_tiles[0] = xm
                else:
                    xm = xms_pool.tile([128, KO, F], BF16, tag="xms")
                shot_reduction(i, xe, sacc, xm)

            # ---------------- exchange prototype partials ----------------
            with tc.high_priority():
                ar_in = dram.tile([128, KO * WAY], BF16, tag="ar_in")
                ar_out = dram.tile([N_CORES, 128, KO * WAY], BF16, tag="ar_out")
                nc.sync.dma_start(ar_in, proto_bf)
                if kind == "skip":
                    for r in range(N_CORES):
                        nc.gpsimd.dma_start(ar_out[r], ar_in[:])
                else:
                    nc.gpsimd.collective_compute(
                        "AllGather",
                        mybir.AluOpType.bypass,
                        replica_groups=[list(range(N_CORES))],
                        ins=[ar_in[:].opt()],
                        outs=[ar_out[:].opt()],
                    )

            # slot 3's query-side phase-1 work (discarded on core 0).
            # Query-side DVE muls are clock-pinned past the prototype chain
            # so they cannot wedge into its DVE window (ample slack: stage_a
            # only needs them at the collective's end).
            with tc.tile_wait_until(QMUL_WAIT_MS):
                query_post(0, xm_tiles[0])

            # ---------------- queries phase 1 (slots 4..12) ----------------
            for j in range(1, NQ_SLOTS):
                i = 3 + j
                psum_t = conv_fp8(i)
                xe = exp_mask(i, psum_t)
                xm = xmq_pool.tile([128, KO, F], BF16, tag="xmq")
                xm_tiles[j] = xm
                with tc.tile_wait_until(QMUL_WAIT_MS):
                    nc.vector.tensor_mul(xm, x_bf_tiles[i], xe)
                    query_post(j, xm)

            # rq = exp(-0.5*ln(|q|^2) + ln 10) = 10/|q| for all queries —
            # computed during the collective window (only needs phase-1 data)
            rq = work.tile([128, NQ_SLOTS, 2], F32, tag="rq")
            nc.scalar.activation(rq, qn_ln, EXP, bias=c_ln10, scale=-0.5)

            # ---------------- consume AllGather result ----------------
            with tc.high_priority():
                protoAll = singles.tile([128, N_CORES, KO * WAY], BF16)
                nc.sync.dma_start(protoAll, ar_out.rearrange("r p kw -> p r kw"))
                pa4 = work.tile([128, 4, KO * WAY], BF16, tag="pa4")
                nc.vector.tensor_add(pa4, protoAll[:, 0:4], protoAll[:, 4:8])
                pa2 = work.tile([128, 2, KO * WAY], BF16, tag="pa2")
                nc.vector.tensor_add(pa2, pa4[:, 0:2], pa4[:, 2:4])
                nc.vector.tensor_add(
                    s_hat.rearrange("p k w -> p (k w)"), pa2[:, 0], pa2[:, 1]
                )
            # 1/||proto|| (rsnb) is applied post-transpose, off the critical
            # path of the score matmuls.
            protosq = work.tile([128, KO, WAY], F32, tag="protosq")
            nc.vector.tensor_mul(protosq, s_hat, s_hat)
            psum_sn = pscratch.tile([128, F], F32, tag="scratch")
            for k in range(KO):
                nc.tensor.matmul(
                    psum_sn[:, :WAY],
                    onesC_f32,
                    protosq[:, k, :],
                    start=(k == 0),
                    stop=(k == KO - 1),
                )
            snln = work.tile([128, WAY], F32, tag="snln")
            nc.scalar.activation(snln, psum_sn[:, :WAY], LN, bias=c_eps)
            nc.scalar.activation(rsnb, snln, EXP, scale=-0.5)

            # ---------------- phase 2: scores + class softmax --------------
            def stage_a(pack):
                """scores already transposed: out[f, class] = xm8^T @ s_hat8"""
                for p2 in range(2):
                    j = 2 * pack + p2
                    for h in range(2):
                        for k in range(KO):
                            nc.tensor.matmul(
                                psum_scores[:, pack, p2, h, :],
                                xm_tiles[j][:, k, 128 * h : 128 * (h + 1)],
                                s_hat[:, k, :],
                                start=(k == 0),
                                stop=(k == KO - 1),
                            )

            for pack in range(NPACKS):
                stage_a(pack)

            # ---------------- batched class softmax over all packs ---------
            # 3 free dims max per ISA op: flatten (pack, p2, h) -> 20 rows
            NR = NPACKS * 4
            L = work.tile([128, NR, WAY], BF16, tag="L")
            nc.vector.tensor_tensor(
                L,
                psum_scores.rearrange("p a q h m -> p (a q h) m"),
                rq.rearrange("p q h -> p (q h)")[:, :, None].to_broadcast(
                    [128, NR, WAY]
                ),
                MULT,
            )
            LL = work.tile([128, NR, WAY], BF16, tag="LL")
            nc.vector.tensor_tensor(
                LL,
                L,
                rsnb[:, None, :].to_broadcast([128, NR, WAY]),
                MULT,
            )
            E = work.tile([128, NR, WAY], BF16, tag="E")
            nc.scalar.activation(E, LL, EXP)
            D = work.tile([128, NR, 1], F32, tag="D")
            nc.vector.reduce_sum(D, E, axis=mybir.AxisListType.X)
            R = work.tile([128, NR, 1], F32, tag="R")
            nc.vector.reciprocal(R, D)
            nc.vector.tensor_tensor(
                pall.rearrange("p a q h m -> p (a q h) m"),
                E,
                R.to_broadcast([128, NR, WAY]),
                MULT,
            )

            # batched partition-sum over all packs, folding the two spatial
            # halves by accumulating both into the same psum rows
            psO = pscratch.tile([128, F], F32, tag="scratch", name="psO")
            for h in range(2):
                nc.tensor.matmul(
                    psO[:2, : NPACKS * 2 * WAY],
                    onesF_bf,
                    pall[:, :, :, h, :],
                    start=(h == 0),
                    stop=(h == 1),
                )
            out_sb = work.tile([1, NQ_SLOTS * WAY], F32, tag="po_sb")
            nc.any.tensor_copy(out_sb, psO[:1, : NQ_SLOTS * WAY])
            nc.sync.dma_start(out[:], out_sb[0:1, :])

    nc.finalize()
    return nc


_NC_CACHE = {}


def _get_nc():
    if "nc" not in _NC_CACHE:
        _NC_CACHE["nc"] = build_nc()
    return _NC_CACHE["nc"]


SHOTS_PER_CORE = [4, 3, 3, 3, 3, 3, 3, 3]       # sums to 25
QUERIES_PER_CORE = [9, 10, 10, 10, 9, 9, 9, 9]  # sums to 75


def _assignments():
    """Per-core (shot global ids, query global ids)."""
    shots = [20 * c + j for c in range(WAY) for j in range(SHOT)]
    queries = [20 * c + SHOT + j for c in range(WAY) for j in range(15)]
    so = np.cumsum([0] + SHOTS_PER_CORE)
    qo = np.cumsum([0] + QUERIES_PER_CORE)
    return [
        (shots[so[k] : so[k + 1]], queries[qo[k] : qo[k + 1]]) for k in range(N_CORES)
    ]


def _core_slot_layout(k):
    """core k: shots fill slots 0..n_s-1, queries fill slots q_start..
    (core 0's 4th shot occupies slot 3; its query j=0 is unused)."""
    n_s = SHOTS_PER_CORE[k]
    q_start = max(n_s, 3)
    return n_s, q_start


def _make_in_maps(x, W, b):
    assert np.all(b == 0.0), "kernel folds b==0 (spec: bias is zeros)"
    wtT = np.ascontiguousarray(W.T)
    wt8 = (wtT * W8_SCALE).astype(ml_dtypes.float8_e4m3)
    x_bf = x.astype(ml_dtypes.bfloat16)
    x_f8 = x.astype(ml_dtypes.float8_e4m3)
    assign = _assignments()
    in_maps = []
    for k in range(N_CORES):
        s_list, q_list = assign[k]
        n_s, q_start = _core_slot_layout(k)
        xs_core = np.zeros((NSLOTS, C, F), dtype=ml_dtypes.bfloat16)
        xs_core[:n_s] = x_bf[s_list]
        xs_core[q_start : q_start + len(q_list)] = x_bf[q_list]
        xs8_core = np.zeros((NSLOTS, C, F), dtype=ml_dtypes.float8_e4m3)
        xs8_core[:n_s] = x_f8[s_list]
        xs8_core[q_start : q_start + len(q_list)] = x_f8[q_list]
        sw_core = np.zeros((NSHOT_SLOTS, WAY), dtype=np.float32)
        for slot, g in enumerate(s_list):
            sw_core[slot, g // 20] = 1.0
        sw_b = np.broadcast_to(
            sw_core[:, None, :], (NSHOT_SLOTS, 128, WAY)
        ).astype(np.float32)
        in_maps.append(
            {
                "xs": xs_core,
                "xs8": xs8_core,
                "wt8": wt8,
                "sw": np.ascontiguousarray(sw_b),
            }
        )
    return in_maps


def kernel(x, W, b):
    x = np.asarray(x, dtype=np.float32).reshape(100, C, F)
    W = np.asarray(W, dtype=np.float32)
    b = np.asarray(b, dtype=np.float32)

    nc = _get_nc()
    in_maps = _make_in_maps(x, W, b)
    res = run_bass_kernel_spmd(nc, in_maps, core_ids=list(range(N_CORES)))

    assign = _assignments()
    final = np.zeros((75, WAY), dtype=np.float32)
    for k in range(N_CORES):
        out_core = np.asarray(res.results[k]["out"], dtype=np.float32).reshape(
            NQ_SLOTS, WAY
        )
        _, q_list = assign[k]
        n_s, q_start = _core_slot_layout(k)
        for slot, g in enumerate(q_list):
            c, j = divmod(g, 20)
            final[15 * c + (j - SHOT)] = out_core[q_start - 3 + slot]
    return final


# revision 47
# speedup vs baseline: 1.0539x; 1.0224x over previous
"""Trainium2 Bass kernel for the few-shot knn-attention module.

Pipeline per sample (512 ch, 16x16 spatial):
  mask = softmax_{c,h,w}(W @ x); xm = x * mask  (mask kept unnormalized on
  device; the softmax denominator is folded algebraically; b==0 by spec)
  prototypes s = mean over 5 shots+space of xm; queries scored by cosine
  similarity against s; softmax over classes; mean over space -> (75, 5).

Distribution: data-parallel over the 100 samples on 8 NeuronCores with 13
slots/core (3 shot slots + 1 shared shot/query slot + 9 query slots,
zero-padded).  All 1x1 convs run in fp8 DoubleRow (W pre-scaled by 16,
folded back in the exp); x stays bf16 for the mask product so the
cosine path keeps full precision.  The prototype partial sums are
exchanged with an AllGather + local tree-sum (modeled ~1.8x cheaper
than AllReduce); the score matmuls are emitted "pre-transposed"
(stationary=xm chunk, moving=prototypes) so scores and |q|^2 land in
[spatial-position, class] orientation directly - no PSUM->SBUF copies
or PE transposes in the tail.
"""

import numpy as np
import ml_dtypes

import concourse.bass as bass
import concourse.mybir as mybir
import concourse.tile as tile
from concourse import bacc
from concourse.bass_utils import run_bass_kernel_spmd

# Force the act-table chooser onto the one set containing BOTH Exp and Ln
# ("natural_log_exp_and_others") so the kernel pays a single table load
# instead of swapping between the exp-only and ln-only sets (~2.7us each).
import concourse.hw_specs as _hw_specs

_ORIG_GET_ACT_TABLES = _hw_specs.get_activation_tables


def _nl_exp_only_tables(arch):
    t = _ORIG_GET_ACT_TABLES(arch)
    return {
        k: (v if k == "natural_log_exp_and_others" else set()) for k, v in t.items()
    }


bacc.get_activation_tables = _nl_exp_only_tables

N_CORES = 8
WAY = 5
SHOT = 5
C = 512
F = 256  # 16*16
KO = C // 128  # 4 partition tiles of the channel dim
NSHOT_SLOTS = 4   # slots 0..3 feed the prototype partials (slot 3 shared)
NQ_SLOTS = 10     # slots 3..12 are query-capable (slot 3 shared)
NSLOTS = 13
NPACKS = NQ_SLOTS // 2
W8_SCALE = 16.0   # W is pre-scaled by 16 for fp8; folded back in the exp
LN10 = float(np.log(10.0))
QMUL_WAIT_MS = 0.0125  # clock-pin query DVE work past the prototype chain
XBF_WAIT_MS = 0.016   # late x_bf DMAs yield their SP-queue slot to ar_in

F32 = mybir.dt.float32
BF16 = mybir.dt.bfloat16
FP8 = mybir.dt.float8e4
EXP = mybir.ActivationFunctionType.Exp
LN = mybir.ActivationFunctionType.Ln
DR = mybir.MatmulPerfMode.DoubleRow
MULT = mybir.AluOpType.mult
ADD = mybir.AluOpType.add


def build_nc(kind="AllGather"):
    nc = bacc.Bacc(None, target_bir_lowering=False)
    xs = nc.dram_tensor("xs", [NSLOTS, C, F], BF16, kind="ExternalInput")
    xs8 = nc.dram_tensor("xs8", [NSLOTS, C, F], FP8, kind="ExternalInput")
    wt8 = nc.dram_tensor("wt8", [C, C], FP8, kind="ExternalInput")
    sw = nc.dram_tensor("sw", [NSHOT_SLOTS, 128, WAY], F32, kind="ExternalInput")
    out = nc.dram_tensor("out", [1, NQ_SLOTS * WAY], F32, kind="ExternalOutput")

    with tile.TileContext(nc) as tc:
        with (
            tc.tile_pool(name="singles", bufs=1) as singles,
            tc.tile_pool(name="xepool", bufs=3) as xepool,
            tc.tile_pool(name="xmq", bufs=NQ_SLOTS) as xmq_pool,
            tc.tile_pool(name="xms", bufs=2) as xms_pool,
            tc.tile_pool(name="xm2", bufs=3) as xm2_pool,
            tc.tile_pool(name="work", bufs=4) as work,
            tc.tile_pool(name="pconv", bufs=2, space="PSUM") as pconv,
            tc.tile_pool(name="pscratch", bufs=2, space="PSUM") as pscratch,
            tc.tile_pool(name="pscores", bufs=1, space="PSUM") as pscores_pool,
            tc.tile_pool(name="pqn", bufs=1, space="PSUM") as pqn_pool,
            tc.tile_pool(name="dram", bufs=2, space="DRAM") as dram,
        ):
            # ---------------- input DMAs (one queue, latency-ordered) ------
            wt8_sb = singles.tile([128, KO, C], FP8)
            x8_tiles = [singles.tile([128, KO, F], FP8, name=f"x8_{i}")
                        for i in range(NSLOTS)]
            x_bf_tiles = [singles.tile([128, KO, F], BF16, name=f"xbf{i}")
                          for i in range(NSLOTS)]

            def dma_x8(i):
                nc.sync.dma_start(
                    x8_tiles[i], xs8[i].rearrange("(ko p) f -> p ko f", p=128)
                )

            def dma_xbf(i):
                nc.sync.dma_start(
                    x_bf_tiles[i], xs[i].rearrange("(ko p) f -> p ko f", p=128)
                )

            dma_x8(0)
            nc.sync.dma_start(wt8_sb, wt8.rearrange("(ko p) o -> p ko o", p=128))
            for i in range(1, NSHOT_SLOTS):
                dma_x8(i)
            for i in range(NSHOT_SLOTS):
                dma_xbf(i)
            sw_sb = singles.tile([128, NSHOT_SLOTS, WAY], F32)
            nc.sync.dma_start(sw_sb, sw.rearrange("s p m -> p s m"))
            for i in range(NSHOT_SLOTS, NSLOTS):
                dma_x8(i)
            # late bf16 copies are only needed once the (clock-pinned) query
            # muls run; pinning them past the exchange lets the ar_in DMA
            # take the earlier SP-queue slot and launch the collective sooner
            for i in range(NSHOT_SLOTS, NSLOTS):
                with tc.tile_wait_until(XBF_WAIT_MS):
                    dma_xbf(i)

            # ---------------- constants ----------------
            onesC_f32 = singles.tile([128, 128], F32)
            nc.vector.memset(onesC_f32, 1.0)
            onesF_bf = singles.tile([128, 2], BF16)
            nc.vector.memset(onesF_bf, 1.0 / F)
            ones1 = singles.tile([128, 1], BF16)
            nc.vector.memset(ones1, 1.0)
            proto = singles.tile([128, KO, WAY], F32)
            nc.vector.memset(proto, 0.0)
            c_eps = singles.tile([128, 1], F32)
            nc.vector.memset(c_eps, 1e-30)
            c_ln10 = singles.tile([128, 1], F32)
            nc.vector.memset(c_ln10, LN10)
            pall = singles.tile([128, NPACKS, 2, 2, WAY], BF16)
            s_hat = singles.tile([128, KO, WAY], BF16)
            rsnb = singles.tile([128, WAY], F32)

            qn_ln = singles.tile([128, NQ_SLOTS, 2], BF16)
            psum_scores = pscores_pool.tile([128, NPACKS, 2, 2, WAY], F32)
            psum_qn = pqn_pool.tile([128, NQ_SLOTS, 2], F32)

            xm_tiles = [None] * NQ_SLOTS

            # ---------------- per-slot compute ----------------
            def conv_fp8(i):
                """fp8 DoubleRow 1x1 conv for slot i (psum = (16W) @ x)."""
                psum_t = pconv.tile([128, KO, F], F32, tag="conv", name=f"conv8_{i}")
                for oo in range(KO):
                    for h in range(2):
                        nc.tensor.matmul(
                            psum_t[:, oo, :],
                            wt8_sb[:, 2 * h : 2 * h + 2, 128 * oo : 128 * (oo + 1)],
                            x8_tiles[i][:, 2 * h : 2 * h + 2, :],
                            start=(h == 0),
                            stop=(h == 1),
                            perf_mode=DR,
                        )
                return psum_t

            def exp_mask(i, psum_t, sacc=None):
                """exp of the conv logits in one ACT op (scale undoes the x16
                on W); optionally accumulates sum_{ko,f} exp into sacc."""
                xe = xepool.tile([128, KO, F], BF16, tag="xe")
                kw = {}
                if sacc is not None:
                    kw["accum_out"] = sacc
                nc.scalar.activation(
                    xe, psum_t, EXP, scale=1.0 / W8_SCALE, **kw
                )
                return xe

            proto_bf = singles.tile([128, KO * WAY], BF16)

            def shot_reduction(i, xe, sacc, xm):
                """prototype contribution of shot slot i: per-channel masked
                sums (fused product+reduce) scaled by 1/S and the class
                one-hot, accumulated into proto."""
                with tc.high_priority():
                    nc.vector.tensor_mul(xm, x_bf_tiles[i], xe)
                    red = work.tile([128, KO, 1], F32, tag="red")
                    nc.vector.reduce_sum(red, xm, axis=mybir.AxisListType.X)
                    # softmax denominator S = sum_{c,f} exp(logit): sacc has
                    # the per-partition sums; the ones-matmul adds over
                    # partitions and broadcasts the total to all partitions
                    psum_s = pscratch.tile([128, F], F32, tag="scratch")
                    nc.tensor.matmul(
                        psum_s[:, :1], onesC_f32, sacc, start=True, stop=True
                    )
                    rS = work.tile([128, 1], F32, tag="rS")
                    nc.vector.reciprocal(rS, psum_s[:, :1])
                    w5b = work.tile([128, WAY], F32, tag="w5b")
                    nc.vector.tensor_scalar_mul(w5b, sw_sb[:, i, :], rS)
                    # contribution + accumulate on the idle Pool engine so
                    # the DVE queue stays clear for the next shot's sums
                    contrib = work.tile([128, KO, WAY], F32, tag="contrib")
                    nc.vector.tensor_tensor(
                        contrib,
                        red[:, :, 0][:, :, None].to_broadcast([128, KO, WAY]),
                        w5b[:, None, :].to_broadcast([128, KO, WAY]),
                        MULT,
                    )
                    if i < NSHOT_SLOTS - 1:
                        nc.vector.tensor_add(proto, proto, contrib)
                    else:
                        # final shot: emit the bf16 exchange payload directly
                        nc.vector.tensor_add(
                            proto_bf.rearrange("p (k w) -> p k w", k=KO),
                            proto,
                            contrib,
                        )

            def query_post(j, xm):
                """|q(f)|^2 directly in [f-part] orientation: stationary=xm2
                chunk, moving=ones -> out[f, 1]; then its log (phase 1)."""
                xm2 = xm2_pool.tile([128, KO, F], BF16, tag="xm2")
                nc.vector.tensor_mul(xm2, xm, xm)
                for h in range(2):
                    for k in range(KO):
                        nc.tensor.matmul(
                            psum_qn[:, j, h : h + 1],
                            xm2[:, k, 128 * h : 128 * (h + 1)],
                            ones1[:, :1],
                            start=(k == 0),
                            stop=(k == KO - 1),
                        )
                nc.scalar.activation(
                    qn_ln[:, j, :], psum_qn[:, j, :], LN, bias=c_eps
                )

            # ---------------- shot phase (slots 0..3) ----------------
            for i in range(NSHOT_SLOTS):
                psum_t = conv_fp8(i)
                sacc = work.tile([128, 1], F32, tag="sacc")
                xe = exp_mask(i, psum_t, sacc=sacc)
                if i == 3:
                    # shared slot: the masked product doubles as query j=0
                    xm = xmq_pool.tile([128, KO, F], BF16, tag="xmq")
                    xm_tiles[0] = xm
                else:
                    xm = xms_pool.tile([128, KO, F], BF16, tag="xms")
                shot_reduction(i, xe, sacc, xm)

            # ---------------- exchange prototype partials ----------------
            with tc.high_priority():
                ar_in = dram.tile([128, KO * WAY], BF16, tag="ar_in")
                ar_out = dram.tile([N_CORES, 128, KO * WAY], BF16, tag="ar_out")
                nc.sync.dma_start(ar_in, proto_bf)
                if kind == "skip":
                    for r in range(N_CORES):
                        nc.gpsimd.dma_start(ar_out[r], ar_in[:])
                else:
                    nc.gpsimd.collective_compute(
                        "AllGather",
                        mybir.AluOpType.bypass,
                        replica_groups=[list(range(N_CORES))],
                        ins=[ar_in[:].opt()],
                        outs=[ar_out[:].opt()],
                    )

            # slot 3's query-side phase-1 work (discarded on core 0).
            # Query-side DVE muls are clock-pinned past the prototype chain
            # so they cannot wedge into its DVE window (ample slack: stage_a
            # only needs them at the collective's end).
            with tc.tile_wait_until(QMUL_WAIT_MS):
                query_post(0, xm_tiles[0])

            # ---------------- queries phase 1 (slots 4..12) ----------------
            for j in range(1, NQ_SLOTS):
                i = 3 + j
                psum_t = conv_fp8(i)
                xe = exp_mask(i, psum_t)
                xm = xmq_pool.tile([128, KO, F], BF16, tag="xmq")
                xm_tiles[j] = xm
                with tc.tile_wait_until(QMUL_WAIT_MS):
                    nc.vector.tensor_mul(xm, x_bf_tiles[i], xe)
                    query_post(j, xm)

            # rq = exp(-0.5*ln(|q|^2) + ln 10) = 10/|q| for all queries —
            # computed during the collective window (only needs phase-1 data)
            rq = work.tile([128, NQ_SLOTS, 2], F32, tag="rq")
            nc.scalar.activation(rq, qn_ln, EXP, bias=c_ln10, scale=-0.5)

            # ---------------- consume AllGather result ----------------
            with tc.high_priority():
                protoAll = singles.tile([128, N_CORES, KO * WAY], BF16)
                nc.sync.dma_start(protoAll, ar_out.rearrange("r p kw -> p r kw"))
                pa4 = work.tile([128, 4, KO * WAY], BF16, tag="pa4")
                nc.vector.tensor_add(pa4, protoAll[:, 0:4], protoAll[:, 4:8])
                pa2 = work.tile([128, 2, KO * WAY], BF16, tag="pa2")
                nc.vector.tensor_add(pa2, pa4[:, 0:2], pa4[:, 2:4])
                nc.vector.tensor_add(
                    s_hat.rearrange("p k w -> p (k w)"), pa2[:, 0], pa2[:, 1]
                )
            # 1/||proto|| (rsnb) is applied post-transpose, off the critical
            # path of the score matmuls.
            protosq = work.tile([128, KO, WAY], F32, tag="protosq")
            nc.vector.tensor_mul(protosq, s_hat, s_hat)
            psum_sn = pscratch.tile([128, F], F32, tag="scratch")
            for k in range(KO):
                nc.tensor.matmul(
                    psum_sn[:, :WAY],
                    onesC_f32,
                    protosq[:, k, :],
                    start=(k == 0),
                    stop=(k == KO - 1),
                )
            snln = work.tile([128, WAY], F32, tag="snln")
            nc.scalar.activation(snln, psum_sn[:, :WAY], LN, bias=c_eps)
            nc.scalar.activation(rsnb, snln, EXP, scale=-0.5)

            # ---------------- phase 2: scores + class softmax --------------
            def stage_a(pack):
                """scores already transposed: out[f, class] = xm8^T @ s_hat8"""
                for p2 in range(2):
                    j = 2 * pack + p2
                    for h in range(2):
                        for k in range(KO):
                            nc.tensor.matmul(
                                psum_scores[:, pack, p2, h, :],
                                xm_tiles[j][:, k, 128 * h : 128 * (h + 1)],
                                s_hat[:, k, :],
                                start=(k == 0),
                                stop=(k == KO - 1),
                            )

            for pack in range(NPACKS):
                stage_a(pack)

            # ---------------- batched class softmax over all packs ---------
            # 3 free dims max per ISA op: flatten (pack, p2, h) -> 20 rows
            NR = NPACKS * 4
            L = work.tile([128, NR, WAY], BF16, tag="L")
            nc.vector.tensor_tensor(
                L,
                psum_scores.rearrange("p a q h m -> p (a q h) m"),
                rq.rearrange("p q h -> p (q h)")[:, :, None].to_broadcast(
                    [128, NR, WAY]
                ),
                MULT,
            )
            LL = work.tile([128, NR, WAY], BF16, tag="LL")
            nc.vector.tensor_tensor(
                LL,
                L,
                rsnb[:, None, :].to_broadcast([128, NR, WAY]),
                MULT,
            )
            E = work.tile([128, NR, WAY], BF16, tag="E")
            nc.scalar.activation(E, LL, EXP)
            D = work.tile([128, NR, 1], F32, tag="D")
            nc.vector.reduce_sum(D, E, axis=mybir.AxisListType.X)
            R = work.tile([128, NR, 1], F32, tag="R")
            nc.vector.reciprocal(R, D)
            nc.vector.tensor_tensor(
                pall.rearrange("p a q h m -> p (a q h) m"),
                E,
                R.to_broadcast([128, NR, WAY]),
                MULT,
            )

            # batched partition-sum over all packs, folding the two spatial
            # halves by accumulating both into the same psum rows
            psO = pscratch.tile([128, F], F32, tag="scratch", name="psO")
            for h in range(2):
                nc.tensor.matmul(
                    psO[:2, : NPACKS * 2 * WAY],
                    onesF_bf,
                    pall[:, :, :, h, :],
                    start=(h == 0),
                    stop=(h == 1),
                )
            out_sb = work.tile([1, NQ_SLOTS * WAY], F32, tag="po_sb")
            nc.any.tensor_copy(out_sb, psO[:1, : NQ_SLOTS * WAY])
            nc.sync.dma_start(out[:], out_sb[0:1, :])

    nc.finalize()
    return nc


_NC_CACHE = {}


def _get_nc():
    if "nc" not in _NC_CACHE:
        _NC_CACHE["nc"] = build_nc()
    return _NC_CACHE["nc"]


SHOTS_PER_CORE = [4, 3, 3, 3, 3, 3, 3, 3]       # sums to 25
QUERIES_PER_CORE = [9, 10, 10, 10, 9, 9, 9, 9]  # sums to 75


def _assignments():
    """Per-core (shot global ids, query global ids)."""
    shots = [20 * c + j for c in range(WAY) for j in range(SHOT)]
    queries = [20 * c + SHOT + j for c in range(WAY) for j in range(15)]
    so = np.cumsum([0] + SHOTS_PER_CORE)
    qo = np.cumsum([0] + QUERIES_PER_CORE)
    return [
        (shots[so[k] : so[k + 1]], queries[qo[k] : qo[k + 1]]) for k in range(N_CORES)
    ]


def _core_slot_layout(k):
    """core k: shots fill slots 0..n_s-1, queries fill slots q_start..
    (core 0's 4th shot occupies slot 3; its query j=0 is unused)."""
    n_s = SHOTS_PER_CORE[k]
    q_start = max(n_s, 3)
    return n_s, q_start


def _make_in_maps(x, W, b):
    assert np.all(b == 0.0), "kernel folds b==0 (spec: bias is zeros)"
    wtT = np.ascontiguousarray(W.T)
    wt8 = (wtT * W8_SCALE).astype(ml_dtypes.float8_e4m3)
    x_bf = x.astype(ml_dtypes.bfloat16)
    x_f8 = x.astype(ml_dtypes.float8_e4m3)
    assign = _assignments()
    in_maps = []
    for k in range(N_CORES):
        s_list, q_list = assign[k]
        n_s, q_start = _core_slot_layout(k)
        xs_core = np.zeros((NSLOTS, C, F), dtype=ml_dtypes.bfloat16)
        xs_core[:n_s] = x_bf[s_list]
        xs_core[q_start : q_start + len(q_list)] = x_bf[q_list]
        xs8_core = np.zeros((NSLOTS, C, F), dtype=ml_dtypes.float8_e4m3)
        xs8_core[:n_s] = x_f8[s_list]
        xs8_core[q_start : q_start + len(q_list)] = x_f8[q_list]
        sw_core = np.zeros((NSHOT_SLOTS, WAY), dtype=np.float32)
        for slot, g in enumerate(s_list):
            sw_core[slot, g // 20] = 1.0
        sw_b = np.broadcast_to(
            sw_core[:, None, :], (NSHOT_SLOTS, 128, WAY)
        ).astype(np.float32)
        in_maps.append(
            {
                "xs": xs_core,
                "xs8": xs8_core,
                "wt8": wt8,
                "sw": np.ascontiguousarray(sw_b),
            }
        )
    return in_maps


def kernel(x, W, b):
    x = np.asarray(x, dtype=np.float32).reshape(100, C, F)
    W = np.asarray(W, dtype=np.float32)
    b = np.asarray(b, dtype=np.float32)

    nc = _get_nc()
    in_maps = _make_in_maps(x, W, b)
    res = run_bass_kernel_spmd(nc, in_maps, core_ids=list(range(N_CORES)))

    assign = _assignments()
    final = np.zeros((75, WAY), dtype=np.float32)
    for k in range(N_CORES):
        out_core = np.asarray(res.results[k]["out"], dtype=np.float32).reshape(
            NQ_SLOTS, WAY
        )
        _, q_list = assign[k]
        n_s, q_start = _core_slot_layout(k)
        for slot, g in enumerate(q_list):
            c, j = divmod(g, 20)
            final[15 * c + (j - SHOT)] = out_core[q_start - 3 + slot]
    return final
